# revision 15
# baseline (speedup 1.0000x reference)
# Trainium2 Bass kernel for nn_AttentiveLinear.
#
# Math:  y[n,o] = sum_i x[n,i] * W[n,i,o] + b[n,o]
#        W[n,i,o] = (x @ Ww)[n, i*128+o] + bw[i*128+o]
#        b        = x @ Wb + bb
# Expand:
#        y[n,o] = sum_{i,j} x_i x_j A_o[i,j] + (x @ (Wb + BW))[n,o] + bb[o]
# with   A_o[i,j] = Ww[j, i*128+o], BW[i,o] = bw[i*128+o].
#
# Since x (x) x is symmetric only S_o = A_o + A_o^T matters off-diagonal;
# the 128*129/2 distinct products pack into 65 circular-diagonal K-chunks:
#   chunk d (0..64), row r: C_d[r, n] = x[n, r] * x[n, (r+d)%128]
#   V_d[r, o] = S_o[r, (r+d)%128]  (d=1..63) | A_o[r,r] (d=0) | S/2 (d=64)
# y^T = sum_d V_d^T @ C_d + lin^T @ x^T + bb: ONE accumulating PE GEMM,
# 65 chunks x 1024 tok/core (~28 us PE) vs baseline's ~110 us.
#
# C production (the verifier forbids partition-offset TensorTensor reads,
# so products at feature-distance d need a cross-partition mover):
#   - d = 0: DVE tensor_tensor xt*xt directly (aligned).
#   - ROT_D chunks: PE rotates xt by d via a permutation matmul into PSUM
#     (stationary = 128-wide slice of one [128,256] two-diagonal tile),
#     ACT casts PSUM->SBUF bf16, DVE multiplies (all base-0 aligned).
#   - the rest: full C_d chunks precomputed on host, DMA-streamed.
# Tuned so PE (~28us + rot), DMA (~3.4MB + 0.26MB/hosted chunk), ACT and
# DVE all land together.

import numpy as np
import ml_dtypes

N_CORES = 8
IN_F = 128
OUT_F = 128
TOK_TOTAL = 8192
TOK = TOK_TOTAL // N_CORES  # 1024 tokens per core
ND = 65  # circular-diagonal chunks

# chunks rotated+multiplied on-chip (PE rot + ACT cast + DVE mult)
ROT_D = list(range(1, 25))
HOSTED_D = [d for d in range(1, ND) if d not in ROT_D]

_CACHE = {}
LAST_RESULT = None


def _chunk_order():
    """Interleave hosted and on-chip chunks so DMA and engines overlap."""
    if not ROT_D or not HOSTED_D:
        return list(range(ND))
    order = []
    hosted = list(HOSTED_D)
    onchip = [0] + list(ROT_D)
    nh, no = len(hosted), len(onchip)
    hi = oi = 0
    for pos in range(ND):
        # spread on-chip chunks evenly across the sequence
        want_onchip = oi * nh <= hi * no if (hi + oi) else True
        if want_onchip and oi < no:
            order.append(onchip[oi])
            oi += 1
        elif hi < nh:
            order.append(hosted[hi])
            hi += 1
        else:
            order.append(onchip[oi])
            oi += 1
    return order


def _build_program():
    import concourse.mybir as mybir
    import concourse.tile as tile
    from concourse import bacc

    dt = mybir.dt
    nc = bacc.Bacc(
        "TRN2", target_bir_lowering=False, debug=False, num_devices=N_CORES
    )

    xt_d = nc.dram_tensor("xt", [IN_F, TOK], dt.bfloat16, kind="ExternalInput")
    # v, cs in chunk-major DRAM layout: each DMA reads a fully contiguous
    # block (128 rows x 2+KB) so HBM streams sequentially.
    v_d = nc.dram_tensor("v", [8 * IN_F, ND * OUT_F // 8], dt.bfloat16,
                         kind="ExternalInput")
    cs_d = nc.dram_tensor(
        "cs", [max(len(HOSTED_D), 1) * IN_F, TOK], dt.bfloat16,
        kind="ExternalInput",
    )
    pm_d = nc.dram_tensor("pm", [IN_F, 2 * IN_F], dt.bfloat16, kind="ExternalInput")
    lin_d = nc.dram_tensor("lin", [IN_F, OUT_F], dt.bfloat16, kind="ExternalInput")
    bbc_d = nc.dram_tensor("bbc", [OUT_F, 1], dt.float32, kind="ExternalInput")
    yt_d = nc.dram_tensor("yt", [OUT_F, TOK], dt.float32, kind="ExternalOutput")

    order = _chunk_order()
    G = TOK // 2  # two PSUM accumulation groups of 512 tokens
    CS_IDX = {d: k for k, d in enumerate(HOSTED_D)}

    with tile.TileContext(nc) as tc:
        with (
            tc.tile_pool(name="const", bufs=1) as const,
            tc.tile_pool(name="cpool", bufs=8) as cpool,
            tc.tile_pool(name="ysb", bufs=1) as ysbp,
            tc.tile_pool(name="psy", bufs=2, space="PSUM") as psyp,
            tc.tile_pool(name="psrot", bufs=2, space="PSUM") as psrot,
        ):
            # ---- input DMAs ----
            xt_s = const.tile([IN_F, TOK], dt.bfloat16)
            nc.sync.dma_start(xt_s[:], xt_d[:])
            lin_s = const.tile([IN_F, OUT_F], dt.bfloat16)
            nc.sync.dma_start(lin_s[:], lin_d[:])
            bbc_s = const.tile([OUT_F, 1], dt.float32)
            nc.sync.dma_start(bbc_s[:], bbc_d[:])
            pm_s = const.tile([IN_F, 2 * IN_F], dt.bfloat16)
            nc.sync.dma_start(pm_s[:], pm_d[:])
            # stationary weights in 8 slices so early chunks unblock fast
            v_s = const.tile([IN_F, ND * OUT_F], dt.bfloat16)
            VSL = ND * OUT_F // 8
            for k in range(8):
                nc.scalar.dma_start(
                    v_s[:, k * VSL : (k + 1) * VSL],
                    v_d[k * IN_F : (k + 1) * IN_F, :],
                )

            # ---- PE warm-up (p-state ramp) while DMAs stream ----
            wps = psrot.tile([IN_F, G], dt.float32)
            for w in range(14):
                nc.tensor.matmul(
                    wps[:, 0:256],
                    xt_s[:, 0:IN_F],
                    xt_s[:, 0:256],
                    start=True,
                    stop=True,
                )

            # ---- linear part opens both accumulation groups ----
            yA = psyp.tile([OUT_F, G], dt.float32)
            yB = psyp.tile([OUT_F, G], dt.float32)
            nc.tensor.matmul(
                yA[:], lin_s[:], xt_s[:, 0:G],
                start=True, stop=False, skip_group_check=True,
            )
            nc.tensor.matmul(
                yB[:], lin_s[:], xt_s[:, G:TOK],
                start=True, stop=False, skip_group_check=True,
            )

            # ---- 65 accumulating chunks ----
            for pos, d in enumerate(order):
                last = pos == ND - 1
                if d in CS_IDX:
                    k = CS_IDX[d]
                    ctile = cpool.tile([IN_F, TOK], dt.bfloat16)
                    # hosted chunk streams straight into the rolling pool;
                    # alternate the two HWDGE rings (SP / ACT)
                    dma_eng = nc.sync if k % 2 == 0 else nc.scalar
                    dma_eng.dma_start(
                        ctile[:], cs_d[k * IN_F : (k + 1) * IN_F, :]
                    )
                    ct = ctile
                elif d == 0:
                    ctile = cpool.tile([IN_F, TOK], dt.bfloat16)
                    nc.vector.tensor_tensor(
                        ctile[:], xt_s[:], xt_s[:], mybir.AluOpType.mult
                    )
                    ct = ctile
                else:
                    # PE rotation: rot[p, n] = xt[(p+d)%128, n], in two
                    # pipelined halves (PSUM tile = 1 bank each)
                    rs = cpool.tile([IN_F, TOK], dt.bfloat16)
                    for h in range(2):
                        hs = slice(h * G, (h + 1) * G)
                        rp = psrot.tile([IN_F, G], dt.float32)
                        nc.tensor.matmul(
                            rp[:], pm_s[:, d : d + IN_F], xt_s[:, hs],
                            start=True, stop=True, skip_group_check=True,
                        )
                        nc.scalar.copy(rs[:, hs], rp[:])  # ACT cast f32->bf16
                    ctile = cpool.tile([IN_F, TOK], dt.bfloat16)
                    nc.vector.tensor_tensor(
                        ctile[:], xt_s[:], rs[:], mybir.AluOpType.mult
                    )
                    ct = ctile
                vsl = v_s[:, d * OUT_F : (d + 1) * OUT_F]
                nc.tensor.matmul(
                    yA[:], vsl, ct[:, 0:G],
                    start=False, stop=last, skip_group_check=True,
                )
                nc.tensor.matmul(
                    yB[:], vsl, ct[:, G:TOK],
                    start=False, stop=last, skip_group_check=True,
                )

            # ---- drain: bias add, PSUM -> SBUF, DMA out ----
            ys = ysbp.tile([OUT_F, TOK], dt.float32)
            nc.scalar.activation(
                ys[:, 0:G], yA[:], mybir.ActivationFunctionType.Identity,
                bias=bbc_s[:],
            )
            nc.sync.dma_start(yt_d[:, 0:G], ys[:, 0:G])
            nc.vector.tensor_scalar_add(ys[:, G:TOK], yB[:], bbc_s[:])
            nc.sync.dma_start(yt_d[:, G:TOK], ys[:, G:TOK])

    nc.compile()
    return nc


def _host_prep(x, Wb, bb, Ww, bw):
    bf16 = ml_dtypes.bfloat16
    x = np.asarray(x, dtype=np.float32)
    Wb = np.asarray(Wb, dtype=np.float32)
    bb = np.asarray(bb, dtype=np.float32)
    Ww = np.asarray(Ww, dtype=np.float32)
    bw = np.asarray(bw, dtype=np.float32)

    # weights: V[r, d*128+o] per the packing above
    A = Ww.reshape(IN_F, IN_F, OUT_F).transpose(2, 1, 0)  # A[o, i, j]
    S = A + A.transpose(0, 2, 1)
    Sp = np.ascontiguousarray(S.transpose(1, 2, 0))  # [r, j, o]
    r = np.arange(IN_F)
    v_host = np.empty((IN_F, ND * OUT_F), dtype=np.float32)
    v_host[:, 0:OUT_F] = A.diagonal(axis1=1, axis2=2).T  # A[o, r, r] -> [r, o]
    for d in range(1, ND):
        vd = Sp[r, (r + d) % IN_F, :]
        if d == 64:
            vd = vd * 0.5
        v_host[:, d * OUT_F : (d + 1) * OUT_F] = vd
    # slice-major DRAM layout matching the 8 v DMAs
    VSL = ND * OUT_F // 8
    v_host = np.ascontiguousarray(
        v_host.astype(bf16).reshape(IN_F, 8, VSL).transpose(1, 0, 2)
    ).reshape(8 * IN_F, VSL)

    # two-diagonal permutation source: pm[k, d+m] = 1 iff m == (k-d)%128,
    # i.e. ones where (col - k) in {0, 128}
    pm = np.zeros((IN_F, 2 * IN_F), dtype=bf16)
    pm[r, r] = 1.0
    pm[r, r + IN_F] = 1.0

    lin = (Wb + bw.reshape(IN_F, OUT_F)).astype(bf16)
    bbc = np.ascontiguousarray(bb.reshape(OUT_F, 1))

    xf = x.reshape(-1, IN_F)
    in_maps = []
    for c in range(N_CORES):
        sh = xf[c * TOK : (c + 1) * TOK]
        xt = np.ascontiguousarray(sh.T).astype(bf16)
        xtf = xt.astype(np.float32)  # products from bf16-rounded x
        cs = np.empty((max(len(HOSTED_D), 1) * IN_F, TOK), dtype=bf16)
        for k, d in enumerate(HOSTED_D):
            cs[k * IN_F : (k + 1) * IN_F, :] = (
                xtf * xtf[(r + d) % IN_F]
            ).astype(bf16)
        in_maps.append(
            {"xt": xt, "v": v_host, "cs": cs, "pm": pm, "lin": lin, "bbc": bbc}
        )
    return in_maps, x.shape


def _ensure_trace_support():
    """If profiling is requested (BASS_TRACE) on an image without
    antenv.axon_hooks, synthesize the hook module so tracing works instead
    of crashing, and keep artifact upload local (no bucket access)."""
    import sys
    import types

    try:
        import antenv

        try:
            from antenv.axon_hooks import get_axon_ntff_profile_hook  # noqa: F401
        except ImportError:
            hook = None
            try:
                from trn_agent_boot.trn_boot import _ntff_profile_via_ctypes

                hook = _ntff_profile_via_ctypes("/opt/axon/libaxon_pjrt.so")
            except Exception:
                pass
            m = types.ModuleType("antenv.axon_hooks")
            hooks = {"h": hook}
            m.get_axon_ntff_profile_hook = lambda: hooks["h"]
            m.set_axon_ntff_profile_hook = lambda h: hooks.__setitem__("h", h)
            sys.modules["antenv.axon_hooks"] = m
            antenv.axon_hooks = m
    except Exception:
        pass
    try:
        import concourse.bass_utils as bu
        from concourse._compat import FishPath

        FishPath.bucket_root()
    except Exception:
        try:
            bu.upload_artifacts = lambda tmpdir: tmpdir
        except Exception:
            pass


def kernel(x, Wb, bb, Ww, bw):
    global LAST_RESULT
    _ensure_trace_support()
    from concourse.bass_utils import run_bass_kernel_spmd

    in_maps, xshape = _host_prep(x, Wb, bb, Ww, bw)
    if "nc" not in _CACHE:
        _CACHE["nc"] = _build_program()
    nc = _CACHE["nc"]

    res = run_bass_kernel_spmd(nc, in_maps, core_ids=list(range(N_CORES)))
    LAST_RESULT = res
    y = np.concatenate(
        [res.results[c]["yt"].T for c in range(N_CORES)], axis=0
    )
    return np.ascontiguousarray(y.reshape(xshape[:-1] + (OUT_F,)), dtype=np.float32)


# revision 19
# speedup vs baseline: 1.4054x; 1.4054x over previous
# Trainium2 Bass kernel for nn_AttentiveLinear.
#
# Math:  y[n,o] = sum_i x[n,i] * W[n,i,o] + b[n,o]
#        W[n,i,o] = (x @ Ww)[n, i*128+o] + bw[i*128+o]
#        b        = x @ Wb + bb
# Expand:
#        y[n,o] = sum_{i,j} x_i x_j A_o[i,j] + (x @ (Wb + BW))[n,o] + bb[o]
# with   A_o[i,j] = Ww[j, i*128+o], BW[i,o] = bw[i*128+o].
#
# Since x (x) x is symmetric only S_o = A_o + A_o^T matters off-diagonal;
# the 128*129/2 distinct products pack into 65 circular-diagonal K-chunks:
#   chunk d (0..64), row r: C_d[r, n] = x[n, r] * x[n, (r+d)%128]
#   V_d[r, o] = S_o[r, (r+d)%128]  (d=1..63) | A_o[r,r] (d=0) | S/2 (d=64)
# y^T = sum_d V_d^T @ C_d + lin^T @ x^T + bb: ONE accumulating PE GEMM,
# 65 chunks x 1024 tok/core (~28 us PE) vs baseline's ~110 us.
#
# C production (the verifier forbids partition-offset TensorTensor reads,
# so products at feature-distance d need a cross-partition mover):
#   - d = 0: DVE tensor_tensor xt*xt directly (aligned).
#   - ROT_D chunks: PE rotates xt by d via a permutation matmul into PSUM
#     (stationary = 128-wide slice of one [128,256] two-diagonal tile),
#     ACT casts PSUM->SBUF bf16, DVE multiplies (all base-0 aligned).
#   - the rest: full C_d chunks precomputed on host, DMA-streamed.
# Tuned so PE (~28us + rot), DMA (~3.4MB + 0.26MB/hosted chunk), ACT and
# DVE all land together.

import numpy as np
import ml_dtypes

N_CORES = 8
IN_F = 128
OUT_F = 128
TOK_TOTAL = 8192
TOK = TOK_TOTAL // N_CORES  # 1024 tokens per core
ND = 65  # circular-diagonal chunks

# chunks rotated+multiplied on-chip (PE rot + ACT cast + DVE mult)
ROT_D = list(range(1, 25))
HOSTED_D = [d for d in range(1, ND) if d not in ROT_D]

_CACHE = {}
LAST_RESULT = None


def _chunk_order():
    """Interleave hosted and on-chip chunks so DMA and engines overlap.
    Lead with a few hosted chunks so the DMA stream builds a runway while
    the PE warms up."""
    if not ROT_D or not HOSTED_D:
        return list(range(ND))
    hosted = list(HOSTED_D)
    onchip = [0] + list(ROT_D)
    LEAD = 4
    order = hosted[:LEAD]
    hosted = hosted[LEAD:]
    nh, no = len(hosted), len(onchip)
    hi = oi = 0
    for pos in range(ND - LEAD):
        want_onchip = oi * nh <= hi * no if (hi + oi) else True
        if want_onchip and oi < no:
            order.append(onchip[oi])
            oi += 1
        elif hi < nh:
            order.append(hosted[hi])
            hi += 1
        else:
            order.append(onchip[oi])
            oi += 1
    return order


def _build_program():
    import concourse.mybir as mybir
    import concourse.tile as tile
    from concourse import bacc

    dt = mybir.dt
    nc = bacc.Bacc(
        "TRN2", target_bir_lowering=False, debug=False, num_devices=N_CORES
    )

    xt_d = nc.dram_tensor("xt", [IN_F, TOK], dt.bfloat16, kind="ExternalInput")
    # v, cs in chunk-major DRAM layout: each DMA reads a fully contiguous
    # block (128 rows x 2+KB) so HBM streams sequentially.
    v_d = nc.dram_tensor("v", [8 * IN_F, ND * OUT_F // 8], dt.bfloat16,
                         kind="ExternalInput")
    cs_d = nc.dram_tensor(
        "cs", [max(len(HOSTED_D), 1) * IN_F, TOK], dt.bfloat16,
        kind="ExternalInput",
    )
    pm_d = nc.dram_tensor("pm", [IN_F, 2 * IN_F], dt.bfloat16, kind="ExternalInput")
    lin_d = nc.dram_tensor("lin", [IN_F, OUT_F], dt.bfloat16, kind="ExternalInput")
    bbc_d = nc.dram_tensor("bbc", [OUT_F, 1], dt.float32, kind="ExternalInput")
    yt_d = nc.dram_tensor("yt", [OUT_F, TOK], dt.float32, kind="ExternalOutput")

    order = _chunk_order()
    G = TOK // 2  # two PSUM accumulation groups of 512 tokens
    CS_IDX = {d: k for k, d in enumerate(HOSTED_D)}

    with tile.TileContext(nc) as tc:
        with (
            tc.tile_pool(name="const", bufs=1) as const,
            tc.tile_pool(name="cpool", bufs=20) as cpool,
            tc.tile_pool(name="rspool", bufs=3) as rspool,
            tc.tile_pool(name="ysb", bufs=1) as ysbp,
            tc.tile_pool(name="psy", bufs=2, space="PSUM") as psyp,
            tc.tile_pool(name="psrot", bufs=2, space="PSUM") as psrot,
        ):
            # ---- input DMAs ----
            xt_s = const.tile([IN_F, TOK], dt.bfloat16)
            nc.sync.dma_start(xt_s[:], xt_d[:])
            lin_s = const.tile([IN_F, OUT_F], dt.bfloat16)
            nc.sync.dma_start(lin_s[:], lin_d[:])
            bbc_s = const.tile([OUT_F, 1], dt.float32)
            nc.sync.dma_start(bbc_s[:], bbc_d[:])
            pm_s = const.tile([IN_F, 2 * IN_F], dt.bfloat16)
            nc.sync.dma_start(pm_s[:], pm_d[:])
            # stationary weights in 8 slices so early chunks unblock fast
            v_s = const.tile([IN_F, ND * OUT_F], dt.bfloat16)
            VSL = ND * OUT_F // 8
            for k in range(8):
                nc.scalar.dma_start(
                    v_s[:, k * VSL : (k + 1) * VSL],
                    v_d[k * IN_F : (k + 1) * IN_F, :],
                )

            # ---- PE warm-up (p-state ramp) while DMAs stream ----
            wps = psrot.tile([IN_F, G], dt.float32)
            for w in range(14):
                nc.tensor.matmul(
                    wps[:, 0:256],
                    xt_s[:, 0:IN_F],
                    xt_s[:, 0:256],
                    start=True,
                    stop=True,
                )

            # ---- linear part opens both accumulation groups ----
            yA = psyp.tile([OUT_F, G], dt.float32)
            yB = psyp.tile([OUT_F, G], dt.float32)
            nc.tensor.matmul(
                yA[:], lin_s[:], xt_s[:, 0:G],
                start=True, stop=False, skip_group_check=True,
            )
            nc.tensor.matmul(
                yB[:], lin_s[:], xt_s[:, G:TOK],
                start=True, stop=False, skip_group_check=True,
            )

            # ---- 65 accumulating chunks ----
            for pos, d in enumerate(order):
                last = pos == ND - 1
                if d in CS_IDX:
                    k = CS_IDX[d]
                    ctile = cpool.tile([IN_F, TOK], dt.bfloat16)
                    # hosted chunk streams straight into the rolling pool;
                    # sync (SP) HWDGE ring is otherwise idle, so triggers
                    # issue far ahead of consumption
                    nc.sync.dma_start(
                        ctile[:], cs_d[k * IN_F : (k + 1) * IN_F, :]
                    )
                    ct = ctile
                elif d == 0:
                    ctile = cpool.tile([IN_F, TOK], dt.bfloat16)
                    nc.vector.tensor_tensor(
                        ctile[:], xt_s[:], xt_s[:], mybir.AluOpType.mult
                    )
                    ct = ctile
                else:
                    # PE rotation: rot[p, n] = xt[(p+d)%128, n], in two
                    # pipelined halves (PSUM tile = 1 bank each)
                    rs = rspool.tile([IN_F, TOK], dt.bfloat16)
                    for h in range(2):
                        hs = slice(h * G, (h + 1) * G)
                        rp = psrot.tile([IN_F, G], dt.float32)
                        nc.tensor.matmul(
                            rp[:], pm_s[:, d : d + IN_F], xt_s[:, hs],
                            start=True, stop=True, skip_group_check=True,
                        )
                        nc.scalar.copy(rs[:, hs], rp[:])  # ACT cast f32->bf16
                    ctile = cpool.tile([IN_F, TOK], dt.bfloat16)
                    nc.vector.tensor_tensor(
                        ctile[:], xt_s[:], rs[:], mybir.AluOpType.mult
                    )
                    ct = ctile
                vsl = v_s[:, d * OUT_F : (d + 1) * OUT_F]
                nc.tensor.matmul(
                    yA[:], vsl, ct[:, 0:G],
                    start=False, stop=last, skip_group_check=True,
                )
                nc.tensor.matmul(
                    yB[:], vsl, ct[:, G:TOK],
                    start=False, stop=last, skip_group_check=True,
                )

            # ---- drain: bias add, PSUM -> SBUF, DMA out ----
            ys = ysbp.tile([OUT_F, TOK], dt.float32)
            nc.scalar.activation(
                ys[:, 0:G], yA[:], mybir.ActivationFunctionType.Identity,
                bias=bbc_s[:],
            )
            nc.sync.dma_start(yt_d[:, 0:G], ys[:, 0:G])
            nc.vector.tensor_scalar_add(ys[:, G:TOK], yB[:], bbc_s[:])
            nc.sync.dma_start(yt_d[:, G:TOK], ys[:, G:TOK])

    nc.compile()
    return nc


def _host_prep(x, Wb, bb, Ww, bw):
    bf16 = ml_dtypes.bfloat16
    x = np.asarray(x, dtype=np.float32)
    Wb = np.asarray(Wb, dtype=np.float32)
    bb = np.asarray(bb, dtype=np.float32)
    Ww = np.asarray(Ww, dtype=np.float32)
    bw = np.asarray(bw, dtype=np.float32)

    # weights: V[r, d*128+o] per the packing above
    A = Ww.reshape(IN_F, IN_F, OUT_F).transpose(2, 1, 0)  # A[o, i, j]
    S = A + A.transpose(0, 2, 1)
    Sp = np.ascontiguousarray(S.transpose(1, 2, 0))  # [r, j, o]
    r = np.arange(IN_F)
    v_host = np.empty((IN_F, ND * OUT_F), dtype=np.float32)
    v_host[:, 0:OUT_F] = A.diagonal(axis1=1, axis2=2).T  # A[o, r, r] -> [r, o]
    for d in range(1, ND):
        vd = Sp[r, (r + d) % IN_F, :]
        if d == 64:
            vd = vd * 0.5
        v_host[:, d * OUT_F : (d + 1) * OUT_F] = vd
    # slice-major DRAM layout matching the 8 v DMAs
    VSL = ND * OUT_F // 8
    v_host = np.ascontiguousarray(
        v_host.astype(bf16).reshape(IN_F, 8, VSL).transpose(1, 0, 2)
    ).reshape(8 * IN_F, VSL)

    # two-diagonal permutation source: pm[k, d+m] = 1 iff m == (k-d)%128,
    # i.e. ones where (col - k) in {0, 128}
    pm = np.zeros((IN_F, 2 * IN_F), dtype=bf16)
    pm[r, r] = 1.0
    pm[r, r + IN_F] = 1.0

    lin = (Wb + bw.reshape(IN_F, OUT_F)).astype(bf16)
    bbc = np.ascontiguousarray(bb.reshape(OUT_F, 1))

    xf = x.reshape(-1, IN_F)
    in_maps = []
    for c in range(N_CORES):
        sh = xf[c * TOK : (c + 1) * TOK]
        xt = np.ascontiguousarray(sh.T).astype(bf16)
        xtf = xt.astype(np.float32)  # products from bf16-rounded x
        cs = np.empty((max(len(HOSTED_D), 1) * IN_F, TOK), dtype=bf16)
        for k, d in enumerate(HOSTED_D):
            cs[k * IN_F : (k + 1) * IN_F, :] = (
                xtf * xtf[(r + d) % IN_F]
            ).astype(bf16)
        in_maps.append(
            {"xt": xt, "v": v_host, "cs": cs, "pm": pm, "lin": lin, "bbc": bbc}
        )
    return in_maps, x.shape


def _ensure_trace_support():
    """If profiling is requested (BASS_TRACE) on an image without
    antenv.axon_hooks, synthesize the hook module so tracing works instead
    of crashing, and keep artifact upload local (no bucket access)."""
    import sys
    import types

    try:
        import antenv

        try:
            from antenv.axon_hooks import get_axon_ntff_profile_hook  # noqa: F401
        except ImportError:
            hook = None
            try:
                from trn_agent_boot.trn_boot import _ntff_profile_via_ctypes

                hook = _ntff_profile_via_ctypes("/opt/axon/libaxon_pjrt.so")
            except Exception:
                pass
            m = types.ModuleType("antenv.axon_hooks")
            hooks = {"h": hook}
            m.get_axon_ntff_profile_hook = lambda: hooks["h"]
            m.set_axon_ntff_profile_hook = lambda h: hooks.__setitem__("h", h)
            sys.modules["antenv.axon_hooks"] = m
            antenv.axon_hooks = m
    except Exception:
        pass
    try:
        import concourse.bass_utils as bu
        from concourse._compat import FishPath

        FishPath.bucket_root()
    except Exception:
        try:
            bu.upload_artifacts = lambda tmpdir: tmpdir
        except Exception:
            pass


def kernel(x, Wb, bb, Ww, bw):
    global LAST_RESULT
    _ensure_trace_support()
    from concourse.bass_utils import run_bass_kernel_spmd

    in_maps, xshape = _host_prep(x, Wb, bb, Ww, bw)
    if "nc" not in _CACHE:
        _CACHE["nc"] = _build_program()
    nc = _CACHE["nc"]

    res = run_bass_kernel_spmd(nc, in_maps, core_ids=list(range(N_CORES)))
    LAST_RESULT = res
    y = np.concatenate(
        [res.results[c]["yt"].T for c in range(N_CORES)], axis=0
    )
    return np.ascontiguousarray(y.reshape(xshape[:-1] + (OUT_F,)), dtype=np.float32)
